# revision 1
# baseline (speedup 1.0000x reference)
"""Causal self-attention (B=4, T=2048, D=1024, H=16) on 8 Trainium2 NeuronCores.

Sharding: batch x head-group hybrid. Core c handles batch b = c % 4 and head
group g = c // 4 (heads 8g..8g+7). Each core computes its heads' attention and
a partial output projection [T, D]; the host sums the two head-group partials
per batch (the all-reduce of the output projection, done at gather time).

Per-core kernel (all matmuls in float32r: ~1.5e-4 rms rel err, 4x fp32 rate):
  1. QKV projection from xT [D, T]: qT/kT produced channel-major [ch, T]
     (heads packed in 64-partition halves), V produced token-major with an
     appended ones*mask column for the softmax denominator.
  2. Attention transposed: scoresT[k,q] = kT.T @ qT (K=64 contraction, even/odd
     head pair packed into PE row groups 0-1/2-3 for concurrency), exp on
     ScalarE (scale=1/8 fused), causal staircase masks multiplied on diagonal
     128x512 blocks only, then attn_outT[ch,q] = v_aug.T @ expT accumulated
     over k tiles. Row 64 of the AV output is the softmax denominator;
     reciprocal + gpsimd partition-broadcast + multiply normalizes.
  3. Output projection per 512-token chunk: out[tok, :] += attnT.T @ wpT.
"""

import sys
import types

import numpy as np


def _ensure_axon_hooks_stub():
    # bass_utils imports antenv.axon_hooks when tracing is requested (e.g. via
    # a BASS_TRACE env); the module is absent in this image. Provide a stub
    # that reports "no hook" unless a harness already installed a real one.
    if "antenv.axon_hooks" in sys.modules:
        return
    mod = types.ModuleType("antenv.axon_hooks")
    _hook = [None]
    mod.set_axon_ntff_profile_hook = lambda h: _hook.__setitem__(0, h)
    mod.get_axon_ntff_profile_hook = lambda: _hook[0]
    sys.modules["antenv.axon_hooks"] = mod
    try:
        import antenv

        antenv.axon_hooks = mod
    except ImportError:
        pass


_ensure_axon_hooks_stub()

import concourse.bass as bass  # noqa: E402
import concourse.mybir as mybir  # noqa: E402
import concourse.tile as tile  # noqa: E402
from concourse import bacc  # noqa: E402
from concourse.bass import ts  # noqa: E402
from concourse.bass_utils import run_bass_kernel_spmd  # noqa: E402

P = 128
B, T, D = 4, 2048, 1024
H, HD = 16, 64
HG = 8          # heads per group (per core)
DG = HG * HD    # 512 channels per group
KO = D // P     # 8 contraction chunks for the projections
TQ = 512        # q tile (matmul free dim)
NQT = T // TQ   # 4
NKT = T // P    # 16 k tiles
F32 = mybir.dt.float32
F32R = mybir.dt.float32r

_PROGRAM = None


def _build_program():
    nc = bacc.Bacc(None, target_bir_lowering=False, debug=False)

    xT = nc.dram_tensor("xT", [D, T], F32R, kind="ExternalInput")
    wqT = nc.dram_tensor("wqT", [D, DG], F32R, kind="ExternalInput")
    wkT = nc.dram_tensor("wkT", [D, DG], F32R, kind="ExternalInput")
    wvT = nc.dram_tensor("wvT", [D, DG], F32R, kind="ExternalInput")
    wpT = nc.dram_tensor("wpT", [DG, D], F32R, kind="ExternalInput")
    dmask = nc.dram_tensor("dmask", [4, P, TQ], F32R, kind="ExternalInput")
    amask = nc.dram_tensor("amask", [P, NKT], F32, kind="ExternalInput")
    out = nc.dram_tensor("out", [T, D], F32, kind="ExternalOutput")

    xT3 = xT.ap().rearrange("(ko p) t -> p ko t", p=P)
    wq3 = wqT.ap().rearrange("(ko p) c -> p ko c", p=P)
    wk3 = wkT.ap().rearrange("(ko p) c -> p ko c", p=P)
    wv3 = wvT.ap().rearrange("(ko p) c -> p ko c", p=P)
    wp3 = wpT.ap().rearrange("(co p) d -> p co d", p=P)

    with tile.TileContext(nc) as tc:
        with tc.tile_pool(name="const", bufs=1) as cpool, \
             tc.tile_pool(name="persist", bufs=1) as perm:
            dmask_sb = cpool.tile([P, 4, TQ], F32R, tag="dmask")
            for o in range(4):
                nc.sync.dma_start(dmask_sb[:, o], dmask.ap()[o])
            amask_sb = cpool.tile([P, NKT], F32, tag="amask")
            nc.sync.dma_start(amask_sb[:], amask.ap())

            # Persistent activations (f32r so they can feed matmuls directly).
            qgT = perm.tile([P, NQT, T], F32R, tag="qgT")   # head pair hp at [:, hp]
            kgT = perm.tile([P, NQT, T], F32R, tag="kgT")
            vaug = perm.tile([P, HG, NKT, HD + 1], F32R, tag="vaug")

            # ---------------- Phase 1: QKV projection ----------------
            with tc.tile_pool(name="w", bufs=1) as wpool, \
                 tc.tile_pool(name="xp", bufs=2) as xpool, \
                 tc.tile_pool(name="qkps", bufs=4, space="PSUM") as qkps, \
                 tc.tile_pool(name="vps", bufs=2, space="PSUM") as vps:
                wq_sb = wpool.tile([P, KO, DG], F32R, tag="wq")
                wk_sb = wpool.tile([P, KO, DG], F32R, tag="wk")
                wv_sb = wpool.tile([P, KO, DG], F32R, tag="wv")
                for kk in range(KO):
                    nc.sync.dma_start(wq_sb[:, kk], wq3[:, kk])
                    nc.sync.dma_start(wk_sb[:, kk], wk3[:, kk])
                    nc.sync.dma_start(wv_sb[:, kk], wv3[:, kk])

                for tc4 in range(NQT):  # 512-token chunks
                    x_sb = xpool.tile([P, KO, TQ], F32R, tag="x")
                    for kk in range(KO):
                        nc.sync.dma_start(x_sb[:, kk], xT3[:, kk, ts(tc4, TQ)])

                    for w_sb, dstT in ((wq_sb, qgT), (wk_sb, kgT)):
                        for cc in range(NQT):  # 128-channel chunks = head pair
                            ps = qkps.tile([P, TQ], F32, tag="qk")
                            for kk in range(KO):
                                nc.tensor.matmul(
                                    ps[:], w_sb[:, kk, ts(cc, P)], x_sb[:, kk],
                                    start=(kk == 0), stop=(kk == KO - 1),
                                )
                            nc.vector.tensor_copy(dstT[:, cc, ts(tc4, TQ)], ps[:])

                    for tt2 in range(TQ // P):  # 128-token subchunks
                        tt = tc4 * (TQ // P) + tt2
                        ps = vps.tile([P, DG], F32, tag="v")
                        for kk in range(KO):
                            nc.tensor.matmul(
                                ps[:], x_sb[:, kk, ts(tt2, P)], wv_sb[:, kk],
                                start=(kk == 0), stop=(kk == KO - 1),
                            )
                        for h in range(HG):
                            nc.vector.tensor_scalar_mul(
                                vaug[:, h, tt, 0:HD], ps[:, ts(h, HD)],
                                amask_sb[:, tt : tt + 1],
                            )
                            nc.vector.tensor_copy(
                                vaug[:, h, tt, HD : HD + 1],
                                amask_sb[:, tt : tt + 1],
                            )

            # ---------- Phase 2+3: attention + output projection ----------
            with tc.tile_pool(name="wpp", bufs=1) as wpp, \
                 tc.tile_pool(name="attn", bufs=2) as apool, \
                 tc.tile_pool(name="expp", bufs=4) as epool, \
                 tc.tile_pool(name="divp", bufs=4) as dpool, \
                 tc.tile_pool(name="outp", bufs=2) as opool, \
                 tc.tile_pool(name="scps", bufs=2, space="PSUM") as scps, \
                 tc.tile_pool(name="avps", bufs=4, space="PSUM") as avps, \
                 tc.tile_pool(name="ops", bufs=2, space="PSUM") as ops:
                wp_sb = wpp.tile([P, DG // P, D], F32R, tag="wp")
                for co in range(DG // P):
                    nc.sync.dma_start(wp_sb[:, co], wp3[:, co])

                for qt in range(NQT):
                    attn_qt = apool.tile([P, NQT, TQ], F32R, tag="attn")
                    nkt = 4 * (qt + 1)
                    for hp in range(NQT):  # head pairs
                        av = [
                            avps.tile([P, TQ], F32, tag="av", name=f"av{qt}_{hp}_{par}")
                            for par in range(2)
                        ]
                        for kt in range(nkt):
                            ex = [None, None]
                            for par in range(2):  # even/odd head of the pair
                                rows = slice(64 * par, 64 * par + 64)
                                sc = scps.tile([P, TQ], F32, tag="sc")
                                nc.tensor.matmul(
                                    sc[:],
                                    kgT[rows, hp, ts(kt, P)],
                                    qgT[rows, hp, ts(qt, TQ)],
                                    start=True, stop=True,
                                )
                                e = epool.tile([P, TQ], F32R, tag="exp")
                                nc.scalar.activation(
                                    e[:], sc[:],
                                    mybir.ActivationFunctionType.Exp,
                                    scale=0.125,
                                )
                                o = kt - 4 * qt
                                if o >= 0:  # diagonal block: causal staircase
                                    nc.vector.tensor_tensor(
                                        e[:], e[:], dmask_sb[:, o],
                                        mybir.AluOpType.mult,
                                    )
                                ex[par] = e
                            for par in range(2):
                                h = 2 * hp + par
                                nc.tensor.matmul(
                                    av[par][: HD + 1, :],
                                    vaug[:, h, kt, :],
                                    ex[par][:],
                                    start=(kt == 0), stop=(kt == nkt - 1),
                                )
                        for par in range(2):
                            r = dpool.tile([1, TQ], F32, tag="recip")
                            nc.vector.reciprocal(r[:], av[par][HD : HD + 1, :])
                            rb = dpool.tile([HD, TQ], F32, tag="rbcast")
                            nc.gpsimd.partition_broadcast(rb[:], r[:], channels=HD)
                            nc.vector.tensor_tensor(
                                attn_qt[slice(64 * par, 64 * par + 64), hp, :],
                                av[par][0:HD, :], rb[:],
                                mybir.AluOpType.mult,
                            )

                    # output projection for this 512-token chunk
                    for tt2 in range(TQ // P):
                        o_sb = opool.tile([P, D], F32, tag="osb")
                        for nb in range(D // TQ):
                            ps = ops.tile([P, TQ], F32, tag="op")
                            for cc in range(DG // P):
                                nc.tensor.matmul(
                                    ps[:],
                                    attn_qt[:, cc, ts(tt2, P)],
                                    wp_sb[:, cc, ts(nb, TQ)],
                                    start=(cc == 0), stop=(cc == DG // P - 1),
                                )
                            nc.vector.tensor_copy(o_sb[:, ts(nb, TQ)], ps[:])
                        nc.sync.dma_start(
                            out.ap()[ts(qt * (TQ // P) + tt2, P), :], o_sb[:]
                        )

    nc.compile()
    return nc


def _get_program():
    global _PROGRAM
    if _PROGRAM is None:
        _PROGRAM = _build_program()
    return _PROGRAM


def _staircase_masks() -> np.ndarray:
    i = np.arange(P)[:, None]
    j = np.arange(TQ)[None, :]
    return np.stack(
        [(j >= 128 * o + i).astype(np.float32) for o in range(4)]
    )  # [4, 128, 512]


def make_in_maps(x, attention_mask, w_qkv, w_proj):
    x = np.asarray(x, dtype=np.float32)
    attention_mask = np.asarray(attention_mask)
    w_qkv = np.asarray(w_qkv, dtype=np.float32)
    w_proj = np.asarray(w_proj, dtype=np.float32)
    dm = _staircase_masks()
    in_maps = []
    for c in range(8):
        g, b = c // 4, c % 4
        rows = slice(DG * g, DG * g + DG)
        in_maps.append({
            "xT": np.ascontiguousarray(x[b].T),
            "wqT": np.ascontiguousarray(w_qkv[0 * D :][rows].T),
            "wkT": np.ascontiguousarray(w_qkv[1 * D :][rows].T),
            "wvT": np.ascontiguousarray(w_qkv[2 * D :][rows].T),
            "wpT": np.ascontiguousarray(w_proj[:, rows].T),
            "dmask": dm,
            "amask": np.ascontiguousarray(
                attention_mask[b].astype(np.float32).reshape(NKT, P).T
            ),
        })
    return in_maps


def run_spmd(in_maps, **kwargs):
    nc = _get_program()
    return run_bass_kernel_spmd(nc, in_maps, list(range(8)), **kwargs)


def kernel(x, attention_mask, w_qkv, w_proj, n_heads):
    assert int(n_heads) == H
    in_maps = make_in_maps(x, attention_mask, w_qkv, w_proj)
    res = run_spmd(in_maps)
    parts = [res.results[c]["out"] for c in range(8)]
    return np.stack([parts[b] + parts[b + 4] for b in range(B)]).astype(np.float32)


# revision 4
# speedup vs baseline: 1.1441x; 1.1441x over previous
"""Causal self-attention (B=4, T=2048, D=1024, H=16) on 8 Trainium2 NeuronCores.

Sharding: batch x head-group hybrid. Core c handles batch b = c % 4 and head
group g = c // 4 (heads 8g..8g+7). Each core computes its heads' attention and
a partial output projection [T, D]; the host sums the two head-group partials
per batch (the all-reduce of the output projection, done at gather time).

Per-core kernel, all matmuls in float32r (~1.5e-4 rms rel err, 4x fp32 rate):
  The QKV projection for each 512-token chunk is interleaved with the
  attention + output projection of the previous chunk, so the (PE-heavy) QKV
  matmuls fill the pipeline bubbles of the (ScalarE-heavy) softmax and keep
  the PE HAM clock-gate at 2.4 GHz.

  - qT/kT are produced channel-major [ch, T] with head pairs packed in
    64-partition halves; the two K=64 score matmuls of a pair run
    concurrently in PE row groups 0-1 / 2-3.
  - V is produced token-major with an appended ones*mask column, so the AV
    matmul emits the softmax denominator as row 64 of its PSUM output.
  - exp on ScalarE (scale=1/8 fused), causal staircase masks multiplied on
    the diagonal 128x512 blocks only.
  - Normalization: denominator row -> gpsimd partition-broadcast ->
    fast Newton reciprocal on 64 partitions -> multiply.
"""

import sys
import types

import numpy as np


def _ensure_axon_hooks_stub():
    # bass_utils imports antenv.axon_hooks when tracing is requested (e.g. via
    # a BASS_TRACE env); the module is absent in this image. Provide a stub
    # that reports "no hook" unless a harness already installed a real one.
    if "antenv.axon_hooks" in sys.modules:
        return
    mod = types.ModuleType("antenv.axon_hooks")
    _hook = [None]
    mod.set_axon_ntff_profile_hook = lambda h: _hook.__setitem__(0, h)
    mod.get_axon_ntff_profile_hook = lambda: _hook[0]
    sys.modules["antenv.axon_hooks"] = mod
    try:
        import antenv

        antenv.axon_hooks = mod
    except ImportError:
        pass


_ensure_axon_hooks_stub()

import concourse.bass as bass  # noqa: E402
import concourse.mybir as mybir  # noqa: E402
import concourse.tile as tile  # noqa: E402
from concourse import bacc  # noqa: E402
from concourse.bass import ts  # noqa: E402
from concourse.bass_utils import run_bass_kernel_spmd  # noqa: E402

P = 128
B, T, D = 4, 2048, 1024
H, HD = 16, 64
HG = 8          # heads per group (per core)
DG = HG * HD    # 512 channels per group
KO = D // P     # 8 contraction chunks for the projections
TQ = 512        # q tile (attention matmul free dim)
TC = 256        # QKV token chunk (matmul free dim)
NQT = T // TQ   # 4
NKT = T // P    # 16 k tiles
F32 = mybir.dt.float32
F32R = mybir.dt.float32r

_PROGRAM = None


def _emit_qkv_chunk(nc, tc8, x_sb, wq_sb, wk_sb, wv_sb, amask_sb, flow, qg, kgT, vaug):
    """QKV projection for one 256-token chunk tc8 (tokens 256*tc8 ...)."""
    half = tc8 % 2
    for w_sb, dst, dcol in ((wq_sb, qg, half * TC), (wk_sb, kgT, tc8 * TC)):
        for cc in range(4):  # 128-channel chunks = head pairs
            ps = flow.tile([P, TC], F32, tag="flow")
            for kk in range(KO):
                nc.tensor.matmul(
                    ps[:], w_sb[:, kk, ts(cc, P)], x_sb[:, kk],
                    start=(kk == 0), stop=(kk == KO - 1),
                )
            nc.vector.tensor_copy(dst[:, cc, dcol : dcol + TC], ps[:])
    for tt2 in range(TC // P):  # 128-token subchunks
        tt = tc8 * (TC // P) + tt2
        for ch2 in range(2):  # 256-channel halves = 4 heads each
            ps = flow.tile([P, 4, HD], F32, tag="flow")
            for kk in range(KO):
                nc.tensor.matmul(
                    ps.rearrange("p h d -> p (h d)"),
                    x_sb[:, kk, ts(tt2, P)],
                    wv_sb[:, kk, ts(ch2, 4 * HD)],
                    start=(kk == 0), stop=(kk == KO - 1),
                )
            nc.vector.tensor_scalar_mul(
                vaug[:, ts(ch2, 4), tt, 0:HD], ps[:],
                amask_sb[:, tt : tt + 1],
            )
        nc.vector.tensor_copy(
            vaug[:, 0:HG, tt, HD : HD + 1],
            amask_sb[:, tt : tt + 1].to_broadcast([P, HG, 1]),
        )


def _emit_attention(nc, qt, qg, kgT, vaug, dmask_sb, wp_sb, flow, avps,
                    epool, dpool, apool, opool, out):
    """Attention + output projection for 512-token q chunk qt."""
    attn_qt = apool.tile([P, NQT, TQ], F32R, tag="attn")
    nkt = 4 * (qt + 1)
    for hp in range(4):  # head pairs
        av = [
            avps.tile([P, TQ], F32, tag="av", name=f"av{qt}_{hp}_{par}")
            for par in range(2)
        ]
        for kt in range(nkt):
            ex = [None, None]
            for par in range(2):  # even/odd head of the pair
                rows = slice(64 * par, 64 * par + 64)
                sc = flow.tile([P, TQ], F32, tag="flow")
                nc.tensor.matmul(
                    sc[:],
                    kgT[rows, hp, ts(kt, P)],
                    qg[rows, hp, :],
                    start=True, stop=True,
                )
                e = epool.tile([P, TQ], F32R, tag="exp")
                nc.scalar.activation(
                    e[:], sc[:], mybir.ActivationFunctionType.Exp, scale=0.125,
                )
                o = kt - 4 * qt
                if o >= 0:  # diagonal block: causal staircase mask
                    nc.vector.tensor_tensor(
                        e[:], e[:], dmask_sb[:, o], mybir.AluOpType.mult,
                    )
                ex[par] = e
            for par in range(2):
                h = 2 * hp + par
                nc.tensor.matmul(
                    av[par][: HD + 1, :],
                    vaug[:, h, kt, :],
                    ex[par][:],
                    start=(kt == 0), stop=(kt == nkt - 1),
                )
        for par in range(2):
            den = dpool.tile([1, TQ], F32, tag="den")
            nc.vector.tensor_copy(den[:], av[par][HD : HD + 1, :])
            rb = dpool.tile([HD, TQ], F32, tag="rb")
            nc.gpsimd.partition_broadcast(rb[:], den[:], channels=HD)
            rec = dpool.tile([HD, TQ], F32, tag="rec")
            scr = dpool.tile([HD, TQ], F32, tag="scr")
            nc.vector.reciprocal_approx_accurate(rec[:], rb[:], scr[:])
            nc.vector.tensor_tensor(
                attn_qt[slice(64 * par, 64 * par + 64), hp, :],
                av[par][0:HD, :], rec[:],
                mybir.AluOpType.mult,
            )

    # output projection for this 512-token chunk
    for tt2 in range(TQ // P):
        o_sb = opool.tile([P, D], F32, tag="osb")
        for nb in range(D // TQ):
            ps = flow.tile([P, TQ], F32, tag="flow")
            for cc in range(DG // P):
                nc.tensor.matmul(
                    ps[:],
                    attn_qt[:, cc, ts(tt2, P)],
                    wp_sb[:, cc, ts(nb, TQ)],
                    start=(cc == 0), stop=(cc == DG // P - 1),
                )
            nc.vector.tensor_copy(o_sb[:, ts(nb, TQ)], ps[:])
        nc.sync.dma_start(out.ap()[ts(qt * (TQ // P) + tt2, P), :], o_sb[:])


def _build_program():
    nc = bacc.Bacc(None, target_bir_lowering=False, debug=False)

    xT = nc.dram_tensor("xT", [D, T], F32R, kind="ExternalInput")
    wqT = nc.dram_tensor("wqT", [D, DG], F32R, kind="ExternalInput")
    wkT = nc.dram_tensor("wkT", [D, DG], F32R, kind="ExternalInput")
    wvT = nc.dram_tensor("wvT", [D, DG], F32R, kind="ExternalInput")
    wpT = nc.dram_tensor("wpT", [DG, D], F32R, kind="ExternalInput")
    dmask = nc.dram_tensor("dmask", [4, P, TQ], F32R, kind="ExternalInput")
    amask = nc.dram_tensor("amask", [P, NKT], F32, kind="ExternalInput")
    out = nc.dram_tensor("out", [T, D], F32, kind="ExternalOutput")

    xT3 = xT.ap().rearrange("(ko p) t -> p ko t", p=P)
    wq3 = wqT.ap().rearrange("(ko p) c -> p ko c", p=P)
    wk3 = wkT.ap().rearrange("(ko p) c -> p ko c", p=P)
    wv3 = wvT.ap().rearrange("(ko p) c -> p ko c", p=P)
    wp3 = wpT.ap().rearrange("(co p) d -> p co d", p=P)

    with tile.TileContext(nc) as tc:
        with tc.tile_pool(name="const", bufs=1) as cpool, \
             tc.tile_pool(name="persist", bufs=1) as perm, \
             tc.tile_pool(name="w", bufs=1) as wpool, \
             tc.tile_pool(name="xp", bufs=2) as xpool, \
             tc.tile_pool(name="qgp", bufs=2) as qpool, \
             tc.tile_pool(name="attn", bufs=2) as apool, \
             tc.tile_pool(name="expp", bufs=3) as epool, \
             tc.tile_pool(name="divp", bufs=1) as dpool, \
             tc.tile_pool(name="outp", bufs=2) as opool, \
             tc.tile_pool(name="flow", bufs=5, space="PSUM") as flow, \
             tc.tile_pool(name="avps", bufs=3, space="PSUM") as avps:
            dmask_sb = cpool.tile([P, 4, TQ], F32R, tag="dmask")
            for o in range(4):
                nc.sync.dma_start(dmask_sb[:, o], dmask.ap()[o])
            amask_sb = cpool.tile([P, NKT], F32, tag="amask")
            nc.sync.dma_start(amask_sb[:], amask.ap())

            wq_sb = wpool.tile([P, KO, DG], F32R, tag="wq")
            wk_sb = wpool.tile([P, KO, DG], F32R, tag="wk")
            wv_sb = wpool.tile([P, KO, DG], F32R, tag="wv")
            wp_sb = wpool.tile([P, DG // P, D], F32R, tag="wp")
            for kk in range(KO):
                nc.sync.dma_start(wq_sb[:, kk], wq3[:, kk])
                nc.sync.dma_start(wk_sb[:, kk], wk3[:, kk])
                nc.sync.dma_start(wv_sb[:, kk], wv3[:, kk])
            for co in range(DG // P):
                nc.sync.dma_start(wp_sb[:, co], wp3[:, co])

            # Persistent activations (f32r so they can feed matmuls directly).
            kgT = perm.tile([P, NQT, T], F32R, tag="kgT")
            vaug = perm.tile([P, HG, NKT, HD + 1], F32R, tag="vaug")

            for tc4 in range(NQT):
                qg = qpool.tile([P, NQT, TQ], F32R, tag="qg")
                for half in range(2):
                    tc8 = 2 * tc4 + half
                    x_sb = xpool.tile([P, KO, TC], F32R, tag="x")
                    for kk in range(KO):
                        nc.sync.dma_start(x_sb[:, kk], xT3[:, kk, ts(tc8, TC)])
                    _emit_qkv_chunk(nc, tc8, x_sb, wq_sb, wk_sb, wv_sb,
                                    amask_sb, flow, qg, kgT, vaug)
                _emit_attention(nc, tc4, qg, kgT, vaug, dmask_sb, wp_sb,
                                flow, avps, epool, dpool, apool, opool, out)

    nc.compile()
    return nc


def _get_program():
    global _PROGRAM
    if _PROGRAM is None:
        _PROGRAM = _build_program()
    return _PROGRAM


def _staircase_masks() -> np.ndarray:
    i = np.arange(P)[:, None]
    j = np.arange(TQ)[None, :]
    return np.stack(
        [(j >= 128 * o + i).astype(np.float32) for o in range(4)]
    )  # [4, 128, 512]


def make_in_maps(x, attention_mask, w_qkv, w_proj):
    x = np.asarray(x, dtype=np.float32)
    attention_mask = np.asarray(attention_mask)
    w_qkv = np.asarray(w_qkv, dtype=np.float32)
    w_proj = np.asarray(w_proj, dtype=np.float32)
    dm = _staircase_masks()
    in_maps = []
    for c in range(8):
        g, b = c // 4, c % 4
        rows = slice(DG * g, DG * g + DG)
        in_maps.append({
            "xT": np.ascontiguousarray(x[b].T),
            "wqT": np.ascontiguousarray(w_qkv[0 * D :][rows].T),
            "wkT": np.ascontiguousarray(w_qkv[1 * D :][rows].T),
            "wvT": np.ascontiguousarray(w_qkv[2 * D :][rows].T),
            "wpT": np.ascontiguousarray(w_proj[:, rows].T),
            "dmask": dm,
            "amask": np.ascontiguousarray(
                attention_mask[b].astype(np.float32).reshape(NKT, P).T
            ),
        })
    return in_maps


def run_spmd(in_maps, **kwargs):
    nc = _get_program()
    return run_bass_kernel_spmd(nc, in_maps, list(range(8)), **kwargs)


def kernel(x, attention_mask, w_qkv, w_proj, n_heads):
    assert int(n_heads) == H
    in_maps = make_in_maps(x, attention_mask, w_qkv, w_proj)
    res = run_spmd(in_maps)
    parts = [res.results[c]["out"] for c in range(8)]
    return np.stack([parts[b] + parts[b + 4] for b in range(B)]).astype(np.float32)
